# revision 38
# baseline (speedup 1.0000x reference)
"""Trainium2 Bass kernel for nn_EnhancedGNN (PNA-style GNN, 4 layers).

Self-contained: host preprocessing + 8-core SPMD Bass program + runner.

 - PNA pre-MLP is linear => per-edge message = P[dst] + Q[src]; all four
   aggregations (sum / sumsq / max / min) only need per-node tables
   [Q | Q^2] (fp16, 256B rows) gathered per edge.
 - std uses var(P+Q) = E[Q^2] - E[Q]^2 (the P shift cancels), so no
   mean/e2 chain is needed.
 - Nodes are degree-sorted and dealt round-robin to 8 cores (rank i -> core
   i%8), giving balanced per-core edge counts and identical per-tile padded
   degree K[t] across cores (required: one SPMD program).
 - Per layer: dma_gather of CSR slot-chunks (<=1024 idx each, rotating SWDGE
   queues), DVE halving-tree segment reductions (fp16 2x mode), per-node
   scalings on the scalar engine (activation scale=AP), packed 1024-wide CAT
   -> 8 transposes + 8 fp16 matmuls with host-folded weights
   (towers/scalers/lin/BN folded).  Biases ride a ones-row in the lhsT.
 - The NEXT layer's table slice and the pool staging transpose are emitted
   inside the tile loop, so each layer ends directly in table-DMA ->
   AllGather, overlapped with that stage's pooling gathers/reductions.
 - Pooling: per stage each core gathers its own node-major fp16 h table in
   graph order (padded with a -1000 dummy row; exact fp32 sum correction),
   then one AllReduce(add) + one AllReduce(max) and the head.
"""
import os
import sys

sys.path.insert(0, "/opt/trn_rl_repo")

import numpy as np

N, E, G = 20000, 320000, 32
L, T, H, IN = 4, 4, 64, 128
F = H // T
BN_EPS = 1e-5
STD_EPS = 1e-5
NC = 8
OWN = N // NC            # 2500 real nodes per core
TILES = 20
OWNP = TILES * 128       # 2560 padded
NTAB = NC * OWNP         # 20480
CHUNK = 8                # gather chunk slots (1024 idx = 65 ring descs; hard HW cap)
POOL_W = 128             # pool slots per (core, graph)
BIGNEG = -1000.0


# ---------------------------------------------------------------- host prep

def _wrap_idx(idx_flat):
    """int16 idx stream -> SBUF wrapped layout [128, n//16]."""
    w = idx_flat.reshape(-1, 16).T          # [16, n//16]
    return np.tile(w, (8, 1)).astype(np.int16)


def preprocess(inputs):
    edge_index = np.asarray(inputs["edge_index"])
    batch = np.asarray(inputs["batch"]).astype(np.int64)
    src_o = edge_index[0].astype(np.int64)
    dst_o = edge_index[1].astype(np.int64)

    deg = np.bincount(dst_o, minlength=N).astype(np.float32)
    logd = np.log(np.maximum(deg, 1.0) + 1.0)
    avg_log = np.log(deg + 1.0).mean(dtype=np.float32)
    amp = (logd / avg_log).astype(np.float32)

    order = np.argsort(-deg, kind="stable")   # ranks: degree descending
    gid = np.empty(N, np.int64)               # old node id -> padded global id
    ranks = np.arange(N)
    gid[order] = (ranks % NC) * OWNP + ranks // NC

    src_g, dst_g = gid[src_o], gid[dst_o]
    so = np.lexsort((src_g, dst_g))
    ssrc, sdst = src_g[so], dst_g[so]
    starts = np.searchsorted(sdst, np.arange(NTAB))
    ends = np.searchsorted(sdst, np.arange(NTAB) + 1)

    K = []
    for t in range(TILES):
        kmax = 1
        for c in range(NC):
            base = c * OWNP + t * 128
            kmax = max(kmax, int((ends[base:base + 128] - starts[base:base + 128]).max()))
        K.append(kmax)

    cores = []
    for c in range(NC):
        dummy = c * OWNP + OWNP - 1
        idx_stream = []
        padk = np.zeros((128, TILES), np.float32)
        degc = np.zeros((128, TILES), np.float32)
        invdeg = np.ones((128, TILES), np.float32)
        ampc = np.ones((128, TILES), np.float32)
        invamp = np.ones((128, TILES), np.float32)
        maskc = np.zeros((128, TILES), np.float32)
        for t in range(TILES):
            k = K[t]
            tile_idx = np.empty((k, 128), np.int64)
            for p in range(128):
                n = c * OWNP + t * 128 + p
                d = int(ends[n] - starts[n])
                lst = ssrc[starts[n]:ends[n]]
                if d == 0:
                    tile_idx[:, p] = dummy
                    padk[p, t] = k
                else:
                    tile_idx[:d, p] = lst
                    tile_idx[d:, p] = lst[0]
                    padk[p, t] = k - d
                loc = t * 128 + p
                r = loc * NC + c   # global degree rank of this slot
                if loc < OWN and r < N:
                    node = order[r]
                    d0 = deg[node]
                    degc[p, t] = d0
                    invdeg[p, t] = 1.0 / max(d0, 1.0)
                    ampc[p, t] = amp[node]
                    invamp[p, t] = 1.0 / amp[node]
                    maskc[p, t] = 1.0 if d0 > 0 else 0.0
            idx_stream.append(tile_idx.astype(np.int16))
        cores.append(dict(idx=idx_stream, padk=padk, deg=degc, invdeg=invdeg,
                          amp=ampc, invamp=invamp, mask=maskc))

    # pooling: per (core, graph) own local node ids, padded to POOL_W with the
    # dummy local row (OWNP-1, whose table row is BIGNEG)
    pool_idx = np.full((NC, G, POOL_W), OWNP - 1, np.int64)
    pool_padcnt = np.zeros((NC, G), np.float32)
    for c in range(NC):
        own_nodes = order[np.arange(OWN) * NC + c]   # local i -> old node id
        b = batch[own_nodes]
        for g in range(G):
            locs = np.where(b == g)[0]
            assert len(locs) <= POOL_W, f"pool overflow {len(locs)}"
            pool_idx[c, g, :len(locs)] = locs
            pool_padcnt[c, g] = POOL_W - len(locs)

    cnt = np.bincount(batch, minlength=G).astype(np.float32)
    invcnt = np.where(cnt > 0, 1.0 / np.maximum(cnt, 1.0), 0.0).astype(np.float32)
    hasg = (cnt > 0).astype(np.float32)

    x = np.asarray(inputs["x"], np.float32)
    xT = np.zeros((NC, IN, OWNP), np.float16)
    for c in range(NC):
        xT[c, :, :OWN] = x[order[np.arange(OWN) * NC + c]].T.astype(np.float16)

    return dict(cores=cores, K=K, order=order, invcnt=invcnt, hasg=hasg,
                xT=xT, pool_idx=pool_idx, pool_padcnt=pool_padcnt)


def fold_weights(inputs):
    pre_W = np.asarray(inputs["pre_W"], np.float32)
    pre_b = np.asarray(inputs["pre_b"], np.float32)
    post_W = np.asarray(inputs["post_W"], np.float32)
    post_b = np.asarray(inputs["post_b"], np.float32)
    lin_W = np.asarray(inputs["lin_W"], np.float32)
    lin_b = np.asarray(inputs["lin_b"], np.float32)
    bn_gamma = np.asarray(inputs["bn_gamma"], np.float32)
    bn_beta = np.asarray(inputs["bn_beta"], np.float32)
    bn_scale = 1.0 / np.sqrt(1.0 + BN_EPS)

    A_bd = np.zeros((L, H, H), np.float32)
    B_bd = np.zeros((L, H, H), np.float32)
    Wx = np.zeros((L, H, H), np.float32)
    W1 = np.zeros((L, 5 * H, H), np.float32)
    W2 = np.zeros((L, 5 * H, H), np.float32)
    W3 = np.zeros((L, 5 * H, H), np.float32)
    for l in range(L):
        for t in range(T):
            sl = slice(t * F, (t + 1) * F)
            A_bd[l][sl, sl] = pre_W[l, t, :F, :]
            B_bd[l][sl, sl] = pre_W[l, t, F:, :]
            Wx[l][sl, sl] = post_W[l, t, :F, :]
            for kind in range(4):
                Wm1 = post_W[l, t, F + kind * F:F + (kind + 1) * F, :]
                Wm2 = post_W[l, t, 5 * F + kind * F:5 * F + (kind + 1) * F, :]
                Wm3 = post_W[l, t, 9 * F + kind * F:9 * F + (kind + 1) * F, :]
                dstk = [1, 2, 3, 4][kind]  # CAT blocks: [P', M1, MN, MX, STD]
                for (Wm, Wt) in ((Wm1, W1), (Wm2, W2), (Wm3, W3)):
                    Wt[l][dstk * H + t * F:dstk * H + (t + 1) * F, sl] += Wm
                    if kind != 3:  # mean/mn/mx each add P'
                        Wt[l][0 * H + t * F:0 * H + (t + 1) * F, sl] += Wm
    # fold lin + BN into the z matmuls:
    # h_next = relu( (cat@W* + xt@Wx + qb) @ linW * bn_g + lin_b*bn_g + bn_b )
    ABp = np.zeros((L, H + 1, H), np.float32)    # [A ; pre_b] (P side)
    Wxb = np.zeros((L, H + 1, H), np.float32)    # [Wx@M ; bias]
    Wzp = np.zeros((L, 8, 128, H), np.float32)   # packed [W1f|W2f|W3f|0]
    for l in range(L):
        g = bn_scale * bn_gamma[l]
        M = lin_W[l] * g[None, :]
        ABp[l, :H] = A_bd[l]
        ABp[l, H] = pre_b[l].reshape(H)
        Wxb[l, :H] = Wx[l] @ M
        Wxb[l, H] = post_b[l].reshape(H) @ M + lin_b[l] * g + bn_beta[l]
        stack = np.concatenate([W1[l] @ M, W2[l] @ M, W3[l] @ M,
                                np.zeros((64, H), np.float32)], axis=0)  # [1024,H]
        Wzp[l] = stack.reshape(8, 128, H)

    return dict(
        enc_W=np.asarray(inputs["enc_W"], np.float32),
        enc_b=np.asarray(inputs["enc_b"], np.float32),
        ABp=ABp, B_bd=B_bd, Wxb=Wxb, Wzp=Wzp,
        out_W1=np.asarray(inputs["out_W1"], np.float32),
        out_b1=np.asarray(inputs["out_b1"], np.float32),
        out_W2=np.asarray(inputs["out_W2"], np.float32),
        out_b2=np.asarray(inputs["out_b2"], np.float32),
    )


# ---------------------------------------------------------------- bass build

def build_program(K):
    import concourse.bacc as bacc
    import concourse.mybir as mybir
    import concourse.tile as tile
    from concourse.library_config import mlp
    from concourse.masks import make_identity
    from concourse.tile import add_dep_helper

    fp32 = mybir.dt.float32
    fp16 = mybir.dt.float16
    i16 = mybir.dt.int16
    AF = mybir.ActivationFunctionType
    OP = mybir.AluOpType

    KMAX = max(K)
    IDXW = sum(k * 8 for k in K)            # int16 columns of the edge idx stream

    nc = bacc.Bacc("TRN2", target_bir_lowering=False, debug=False,
                   num_devices=NC, num_swdge_queues=4)

    # ------------- I/O
    xT_d = nc.dram_tensor("xT", [IN, OWNP], fp16, kind="ExternalInput")
    eidx_d = nc.dram_tensor("eidx", [128, IDXW], i16, kind="ExternalInput")
    pidx_d = nc.dram_tensor("pidx", [128, G * POOL_W // 16], i16, kind="ExternalInput")
    consts_d = nc.dram_tensor("consts", [128, TILES, 8], fp32, kind="ExternalInput")
    gconst_d = nc.dram_tensor("gconst", [128, 3, G], fp32, kind="ExternalInput")
    encW_d = nc.dram_tensor("encW", [IN, H], fp16, kind="ExternalInput")
    encb_d = nc.dram_tensor("encb", [1, H], fp16, kind="ExternalInput")
    ABp_d = nc.dram_tensor("ABp", [L, H + 1, H], fp16, kind="ExternalInput")
    B_d = nc.dram_tensor("B", [L, H, H], fp16, kind="ExternalInput")
    Wxb_d = nc.dram_tensor("Wxb", [L, H + 1, H], fp16, kind="ExternalInput")
    Wzp_d = nc.dram_tensor("Wzp", [L, 8, 128, H], fp16, kind="ExternalInput")
    hW1_d = nc.dram_tensor("hW1", [3, 5, H, H], fp32, kind="ExternalInput")
    hb1_d = nc.dram_tensor("hb1", [1, H], fp32, kind="ExternalInput")
    hW2_d = nc.dram_tensor("hW2", [H, 1], fp32, kind="ExternalInput")
    negrow_d = nc.dram_tensor("negrow", [1, 128], fp16, kind="ExternalInput")
    hb2_d = nc.dram_tensor("hb2", [1, 1], fp32, kind="ExternalInput")
    out_d = nc.dram_tensor("out", [G, 1], fp32, kind="ExternalOutput")

    # internal DRAM
    slice_d = nc.dram_tensor("slice_d", [OWNP, 2 * H], fp16)
    tables = [nc.dram_tensor(f"table{l}", [NTAB, 2 * H], fp16, addr_space="Shared")
              for l in range(L)]
    hnm_d = nc.dram_tensor("hnm", [OWNP, 5 * 128], fp16)
    psum_in = nc.dram_tensor("psum_in", [128, 5 * G], fp32)
    pmax_in = nc.dram_tensor("pmax_in", [128, 5 * G], fp32)
    psum_out = nc.dram_tensor("psum_out", [128, 5 * G], fp32, addr_space="Shared")
    pmax_out = nc.dram_tensor("pmax_out", [128, 5 * G], fp32, addr_space="Shared")

    with tile.TileContext(nc) as tc:
        with (
            tc.tile_pool(name="persist", bufs=1) as pers,
            tc.tile_pool(name="wpool", bufs=1) as wp,
            tc.tile_pool(name="gat", bufs=4) as gat,
            tc.tile_pool(name="tree", bufs=4) as trp,
            tc.tile_pool(name="nm", bufs=4) as nmp,
            tc.tile_pool(name="cat", bufs=4) as catp,
            tc.tile_pool(name="cs", bufs=6) as csp,
            tc.tile_pool(name="stage", bufs=2) as stp,
            tc.tile_pool(name="hst", bufs=2) as hsp,
            tc.tile_pool(name="ps", bufs=3, space="PSUM") as psp,
            tc.tile_pool(name="psT", bufs=2, space="PSUM") as psT,
            tc.tile_pool(name="psZ", bufs=2, space="PSUM") as psZ,
        ):
            nc.gpsimd.load_library(mlp)

            # ---------- constants / weights
            ident = wp.tile([128, 128], fp32, tag="ident")
            make_identity(nc, ident[:])
            ident16 = wp.tile([128, 128], fp16, tag="ident16")
            nc.vector.tensor_copy(out=ident16[:], in_=ident[:])
            xT = pers.tile([IN, OWNP], fp16, tag="xT")
            nc.sync.dma_start(xT[:], xT_d[:])
            eidx = pers.tile([128, IDXW], i16, tag="eidx")
            nc.sync.dma_start(eidx[:], eidx_d[:])
            pidx = pers.tile([128, G * POOL_W // 16], i16, tag="pidx")
            nc.sync.dma_start(pidx[:], pidx_d[:])
            consts = pers.tile([128, TILES, 8], fp32, tag="consts")
            nc.sync.dma_start(consts[:], consts_d[:])
            gconst = pers.tile([128, 3, G], fp32, tag="gconst")
            nc.sync.dma_start(gconst[:], gconst_d[:])
            encW = wp.tile([IN, H], fp16, tag="encW")
            nc.sync.dma_start(encW[:], encW_d[:])
            encb = wp.tile([1, H], fp16, tag="encb")
            nc.sync.dma_start(encb[:], encb_d[:])
            ABpv = wp.tile([H + 1, L * H], fp16, tag="ABpv")
            nc.sync.dma_start(ABpv[:].rearrange("k (l m) -> k l m", l=L),
                              ABp_d.ap().rearrange("l k m -> k l m"))
            Bv = wp.tile([H, L * H], fp16, tag="Bv")
            nc.sync.dma_start(Bv[:].rearrange("k (l m) -> k l m", l=L),
                              B_d.ap().rearrange("l k m -> k l m"))
            Wxbv = wp.tile([H + 1, L * H], fp16, tag="Wxbv")
            nc.sync.dma_start(Wxbv[:].rearrange("k (l m) -> k l m", l=L),
                              Wxb_d.ap().rearrange("l k m -> k l m"))
            Wzv = wp.tile([128, L * 8 * H], fp16, tag="Wzv")
            nc.sync.dma_start(Wzv[:].rearrange("k (l c m) -> k l c m", l=L, c=8),
                              Wzp_d.ap().rearrange("l c k m -> k l c m"))
            hW1 = wp.tile([H, 15 * H], fp32, tag="hW1")
            nc.sync.dma_start(hW1[:].rearrange("k (a s m) -> k a s m", a=3, s=5),
                              hW1_d.ap().rearrange("a s k m -> k a s m"))
            hb1 = wp.tile([1, H], fp32, tag="hb1")
            nc.sync.dma_start(hb1[:], hb1_d[:])
            hW2 = wp.tile([H, 1], fp32, tag="hW2")
            nc.sync.dma_start(hW2[:], hW2_d[:])
            hb2 = wp.tile([1, 1], fp32, tag="hb2")
            nc.sync.dma_start(hb2[:], hb2_d[:])
            ones_t = wp.tile([1, OWNP], fp16, tag="ones")
            nc.vector.memset(ones_t[:], 1.0)
            eps_t = wp.tile([128, 1], fp32, tag="eps")
            nc.vector.memset(eps_t[:], STD_EPS)
            zero_t = wp.tile([128, 128], fp16, tag="zero")
            nc.vector.memset(zero_t[:], 0.0)

            hbuf = [pers.tile([H + 1, OWNP], fp16, tag=f"h{i}", name=f"hbuf{i}")
                    for i in range(2)]
            for hb in hbuf:
                nc.vector.memset(hb[H:H + 1, :], 1.0)

            pool_s = pers.tile([128, 5, G], fp32, tag="pool_s")
            pool_m = pers.tile([128, 5, G], fp32, tag="pool_m")

            qstate = dict(rot=0)

            def stage_tile_post(h, t, hstage, next_l, tstage, alt):
                """After h[:, tile t] is final: pool transpose + next table."""
                pt = psT.tile([128, H], fp16, tag="T")
                nc.tensor.transpose(out=pt[:], in_=h[0:H, t * 128:(t + 1) * 128],
                                    identity=ident16[0:H, 0:H])
                if alt:
                    nc.scalar.copy(out=hstage[:, t, 0:H], in_=pt[:])
                else:
                    nc.vector.tensor_copy(out=hstage[:, t, 0:H], in_=pt[:])
                if next_l is not None:
                    B_l = Bv[:, next_l * H:(next_l + 1) * H]
                    ps2 = psp.tile([128, H], fp32, tag="ps")
                    nc.tensor.matmul(out=ps2[:], lhsT=h[0:H, t * 128:(t + 1) * 128],
                                     rhs=B_l, start=True, stop=True)
                    nc.scalar.copy(out=tstage[:, t, 0:H], in_=ps2[:])
                    nc.scalar.activation(out=tstage[:, t, H:2 * H], in_=ps2[:],
                                         func=AF.Square)

            pool_wdmas = []

            def stage_flush(stage, next_l, hstage, tstage):
                """Emit table DMA + AllGather (if any); stage the pool table."""
                cc = None
                if next_l is not None:
                    nc.sync.dma_start(slice_d.ap().rearrange("(t p) d -> p t d", p=128),
                                      tstage[:])
                    cc = nc.gpsimd.collective_compute(
                        "AllGather", OP.bypass,
                        replica_groups=[list(range(NC))],
                        ins=[slice_d[:]], outs=[tables[next_l][:]])
                nc.sync.dma_start(hstage[127:128, TILES - 1, :], negrow_d[:])
                wdma = nc.sync.dma_start(
                    hnm_d.ap()[:, stage * 128:(stage + 1) * 128]
                    .rearrange("(t p) d -> p t d", p=128), hstage[:])
                pool_wdmas.append(wdma)
                return cc

            # ---------- encoder (stage 0) + table 0 build interleaved
            h = hbuf[0]
            hstage = hsp.tile([128, TILES, 128], fp16, tag="hstage")
            tstage = stp.tile([128, TILES, 2 * H], fp16, tag="tstage")
            for cchunk in range(5):
                sl = slice(cchunk * 512, (cchunk + 1) * 512)
                ps = psp.tile([H, 512], fp32, tag="ps")
                nc.tensor.matmul(out=ps[:], lhsT=encW[:], rhs=xT[:, sl], start=True, stop=False)
                nc.tensor.matmul(out=ps[:], lhsT=encb[:], rhs=ones_t[:, sl], start=False, stop=True)
                if cchunk % 2 == 0:
                    nc.scalar.copy(out=h[0:H, sl], in_=ps[:])
                else:
                    nc.vector.tensor_copy(out=h[0:H, sl], in_=ps[:])
                if cchunk == 4:
                    nc.vector.memset(h[0:H, OWN:OWNP], 0.0)
                for t in range(cchunk * 4, cchunk * 4 + 4):
                    stage_tile_post(h, t, hstage, 0, tstage, alt=(t % 2 == 0))
            cc = stage_flush(0, 0, hstage, tstage)

            # ---------- layers (stage l+1 consumes tables[l])
            for l in range(L):
                hn = hbuf[(l + 1) % 2]
                ABp_l = ABpv[:, l * H:(l + 1) * H]
                Wxb_l = Wxbv[:, l * H:(l + 1) * H]
                next_l = l + 1 if l + 1 < L else None
                hstage = hsp.tile([128, TILES, 128], fp16, tag="hstage",
                                  name=f"hstage{l}")
                tstage = None
                if next_l is not None:
                    tstage = stp.tile([128, TILES, 2 * H], fp16, tag="tstage",
                                      name=f"tstage{l}")

                qoff = 0
                for t in range(TILES):
                    k = K[t]
                    invdeg_s = consts[:, t, 1:2]
                    amp_s = consts[:, t, 2:3]
                    invamp_s = consts[:, t, 3:4]
                    mask_s = consts[:, t, 4:5]
                    padk_s = consts[:, t, 5:6]

                    gt = gat.tile([128, KMAX, 2 * H], fp16, tag="g")
                    nchunks = (k + CHUNK - 1) // CHUNK
                    for ci in range(nchunks):
                        k0 = ci * CHUNK
                        kc = min(CHUNK, k - k0)
                        gi = nc.gpsimd.dma_gather(
                            gt[:, k0:k0 + kc, :], tables[l][:],
                            eidx[:, (qoff + k0 * 8):(qoff + (k0 + kc) * 8)],
                            kc * 128, kc * 128, 2 * H, queue_num=qstate["rot"] % 4)
                        qstate["rot"] += 1
                        add_dep_helper(gi.ins, cc.ins, reason="allgather->gather")
                    qoff += k * 8

                    CAT = catp.tile([128, 1024], fp16, tag="CAT")

                    def halving(width, op, tag, final_ap):
                        """Tree-reduce gt[:, 0:k, 0:width] -> final_ap [128,1,width]."""
                        if k == 1:
                            nc.vector.tensor_copy(out=final_ap,
                                                  in_=gt[:, 0:1, 0:width])
                            return
                        cur = k
                        buf = None
                        while cur > 2:
                            hh = cur // 2
                            dst = trp.tile([128, (KMAX + 1) // 2, width], fp16, tag=tag)
                            if buf is None:
                                nc.vector.tensor_tensor(
                                    out=dst[:, 0:hh, :], in0=gt[:, 0:hh, 0:width],
                                    in1=gt[:, hh:2 * hh, 0:width], op=op)
                                if cur % 2:
                                    nc.vector.tensor_copy(out=dst[:, hh, :],
                                                          in_=gt[:, cur - 1, 0:width])
                            else:
                                nc.vector.tensor_tensor(
                                    out=dst[:, 0:hh, :], in0=buf[:, 0:hh, :],
                                    in1=buf[:, hh:2 * hh, :], op=op)
                                if cur % 2:
                                    nc.vector.tensor_copy(out=dst[:, hh, :],
                                                          in_=buf[:, cur - 1, :])
                            buf = dst
                            cur = hh + cur % 2
                        if buf is None:
                            nc.vector.tensor_tensor(
                                out=final_ap, in0=gt[:, 0:1, 0:width],
                                in1=gt[:, 1:2, 0:width], op=op)
                        else:
                            nc.vector.tensor_tensor(
                                out=final_ap, in0=buf[:, 0:1, :],
                                in1=buf[:, 1:2, :], op=op)

                    S = nmp.tile([128, 1, 2 * H], fp16, tag="S")
                    halving(2 * H, OP.add, "trs", S[:])
                    halving(H, OP.min, "trn",
                            CAT[:, 2 * H:3 * H].rearrange("p (o w) -> p o w", o=1))
                    halving(H, OP.max, "trm",
                            CAT[:, 3 * H:4 * H].rearrange("p (o w) -> p o w", o=1))

                    # exact-sum correction for padded slots (scalar engine)
                    corr = nmp.tile([128, 2 * H], fp16, tag="corr")
                    nc.scalar.activation(out=corr[:], in_=gt[:, 0, :],
                                         func=AF.Copy, scale=padk_s)
                    Sc = nmp.tile([128, 2 * H], fp32, tag="Sc")
                    nc.vector.tensor_tensor(out=Sc[:], in0=S[:, 0, :], in1=corr[:],
                                            op=OP.subtract)

                    # P node-major ([A;pb] matmul via ones row), masked on ACT
                    pps = psp.tile([128, H], fp32, tag="ps")
                    nc.tensor.matmul(out=pps[:], lhsT=h[0:H + 1, t * 128:(t + 1) * 128],
                                     rhs=ABp_l, start=True, stop=True)
                    nc.scalar.activation(out=CAT[:, 0:H], in_=pps[:],
                                         func=AF.Copy, scale=mask_s)
                    # M1 = E[Q], E2a = E[Q^2]; var = E2a - M1^2 (P cancels)
                    nc.scalar.activation(out=CAT[:, H:2 * H], in_=Sc[:, 0:H],
                                         func=AF.Copy, scale=invdeg_s)
                    msq = nmp.tile([128, H], fp32, tag="msq")
                    nc.scalar.activation(out=msq[:], in_=Sc[:, 0:H],
                                         func=AF.Square, scale=invdeg_s)
                    E2a = nmp.tile([128, H], fp32, tag="E2a")
                    nc.scalar.activation(out=E2a[:], in_=Sc[:, H:2 * H],
                                         func=AF.Copy, scale=invdeg_s)
                    var = nmp.tile([128, H], fp32, tag="var")
                    nc.vector.tensor_tensor(out=var[:], in0=E2a[:], in1=msq[:],
                                            op=OP.subtract)
                    nc.vector.tensor_tensor(out=var[:], in0=var[:],
                                            in1=zero_t[:, 0:H], op=OP.max)
                    nc.scalar.activation(out=CAT[:, 4 * H:5 * H], in_=var[:],
                                         func=AF.Sqrt, bias=eps_t[:])
                    nc.vector.memset(CAT[:, 15 * H:1024], 0.0)
                    nc.scalar.activation(out=CAT[:, 5 * H:10 * H], in_=CAT[:, 0:5 * H],
                                         func=AF.Copy, scale=amp_s)
                    nc.scalar.activation(out=CAT[:, 10 * H:15 * H], in_=CAT[:, 0:5 * H],
                                         func=AF.Copy, scale=invamp_s)

                    # transposes + z matmuls (lin/BN/bias folded, all fp16)
                    zps = psZ.tile([H, 128], fp32, tag="z")
                    nc.tensor.matmul(out=zps[:], lhsT=Wxb_l,
                                     rhs=h[0:H + 1, t * 128:(t + 1) * 128],
                                     start=True, stop=False, skip_group_check=True)
                    for cci in range(8):
                        tp = psT.tile([128, 128], fp16, tag="T")
                        nc.tensor.transpose(out=tp[:],
                                            in_=CAT[:, cci * 128:(cci + 1) * 128],
                                            identity=ident16[:])
                        cs = csp.tile([128, 128], fp16, tag="catS")
                        if cci % 8 < 5:
                            nc.scalar.copy(out=cs[:], in_=tp[:])
                        else:
                            nc.vector.tensor_copy(out=cs[:], in_=tp[:])
                        w_ap = Wzv[:, (l * 8 + cci) * H:(l * 8 + cci + 1) * H]
                        nc.tensor.matmul(out=zps[:], lhsT=w_ap, rhs=cs[:],
                                         start=False, stop=(cci == 7),
                                         skip_group_check=True)
                    nc.vector.tensor_tensor(out=hn[0:H, t * 128:(t + 1) * 128],
                                            in0=zps[:], in1=zero_t[0:H, :], op=OP.max)
                    if t == TILES - 1:
                        nc.vector.memset(hn[0:H, OWN:OWNP], 0.0)
                    stage_tile_post(hn, t, hstage, next_l, tstage, alt=(t % 2 == 0))
                cc = stage_flush(l + 1, next_l, hstage, tstage)
                h = hn

            # ---------- pooling (all 5 stages in one gather round)
            for ch in range(16):
                gp = gat.tile([128, 5, 256], fp16, tag="poolg")
                gi = nc.gpsimd.dma_gather(
                    gp[:], hnm_d[:], pidx[:, ch * 16:(ch + 1) * 16],
                    256, 256, 5 * 128, transpose=True, queue_num=ch % 4)
                for wd in pool_wdmas:
                    add_dep_helper(gi.ins, wd.ins, reason="hnm write->gather")
                gv = gp[:].rearrange("p s (g w) -> p (s g) w", w=POOL_W)
                nc.vector.tensor_reduce(out=pool_s[:, :, ch * 2:(ch + 1) * 2],
                                        in_=gv, axis=mybir.AxisListType.X, op=OP.add)
                nc.vector.tensor_reduce(out=pool_m[:, :, ch * 2:(ch + 1) * 2],
                                        in_=gv, axis=mybir.AxisListType.X, op=OP.max)
            for s in range(5):
                nc.vector.tensor_add(out=pool_s[:, s, :], in0=pool_s[:, s, :],
                                     in1=gconst[:, 0, :])

            # ---------- readout
            w1 = nc.sync.dma_start(psum_in[:], pool_s[:].rearrange("p s g -> p (s g)"))
            w2 = nc.sync.dma_start(pmax_in[:], pool_m[:].rearrange("p s g -> p (s g)"))
            cs1 = nc.gpsimd.collective_compute("AllReduce", OP.add,
                                               replica_groups=[list(range(NC))],
                                               ins=[psum_in[:]], outs=[psum_out[:]])
            cs2 = nc.gpsimd.collective_compute("AllReduce", OP.max,
                                               replica_groups=[list(range(NC))],
                                               ins=[pmax_in[:]], outs=[pmax_out[:]])
            gsum = pers.tile([128, 5, G], fp32, tag="gsum")
            gmax = pers.tile([128, 5, G], fp32, tag="gmax")
            r1 = nc.sync.dma_start(gsum[:], psum_out.ap().rearrange("p (s g) -> p s g", g=G))
            r2 = nc.sync.dma_start(gmax[:], pmax_out.ap().rearrange("p (s g) -> p s g", g=G))
            add_dep_helper(r1.ins, cs1.ins, reason="allreduce->read")
            add_dep_helper(r2.ins, cs2.ins, reason="allreduce->read")
            gmean = pers.tile([128, 5, G], fp32, tag="gmean")
            for s in range(5):
                nc.vector.tensor_mul(out=gmean[:, s, :], in0=gsum[:, s, :],
                                     in1=gconst[:, 1, :])
                nc.vector.tensor_mul(out=gmax[:, s, :], in0=gmax[:, s, :],
                                     in1=gconst[:, 2, :])
            ones_g = wp.tile([1, G], fp32, tag="ones_g")
            nc.vector.memset(ones_g[:], 1.0)
            zp = psp.tile([H, G], fp32, tag="ps")
            first = True
            for kind, buf in ((0, gmean), (1, gsum), (2, gmax)):
                for s in range(5):
                    nc.tensor.matmul(out=zp[:],
                                     lhsT=hW1[:, (kind * 5 + s) * H:(kind * 5 + s + 1) * H],
                                     rhs=buf[0:H, s, :], start=first, stop=False)
                    first = False
            nc.tensor.matmul(out=zp[:], lhsT=hb1[:], rhs=ones_g[:], start=False, stop=True)
            zs = pers.tile([H, G], fp32, tag="zs")
            nc.vector.tensor_scalar_max(out=zs[:], in0=zp[:], scalar1=0.0)
            op_ps = psp.tile([1, G], fp32, tag="ps")
            nc.tensor.matmul(out=op_ps[:], lhsT=hW2[:], rhs=zs[:], start=True, stop=False)
            nc.tensor.matmul(out=op_ps[:], lhsT=hb2[:], rhs=ones_g[:], start=False, stop=True)
            osb = pers.tile([1, G], fp32, tag="osb")
            nc.vector.tensor_copy(out=osb[:], in_=op_ps[:])
            nc.sync.dma_start(out_d.ap().rearrange("g o -> o g"), osb[:])

    nc.compile()
    return nc


# ---------------------------------------------------------------- runner

def kernel(**inputs):
    from concourse.bass_utils import run_bass_kernel_spmd

    pp = preprocess(inputs)
    fw = fold_weights(inputs)
    K = pp["K"]

    nc = build_program(K)

    in_maps = []
    for c in range(NC):
        co = pp["cores"][c]
        eidx_flat = np.concatenate([co["idx"][t].reshape(-1) for t in range(TILES)])
        eidx = _wrap_idx(eidx_flat)
        pidx = _wrap_idx(pp["pool_idx"][c].reshape(-1).astype(np.int16))
        consts = np.zeros((128, TILES, 8), np.float32)
        consts[:, :, 0] = co["deg"]
        consts[:, :, 1] = co["invdeg"]
        consts[:, :, 2] = co["amp"]
        consts[:, :, 3] = co["invamp"]
        consts[:, :, 4] = co["mask"]
        consts[:, :, 5] = co["padk"]
        gconst = np.zeros((128, 3, G), np.float32)
        gconst[:, 0, :] = (-BIGNEG) * pp["pool_padcnt"][c][None, :]
        gconst[:, 1, :] = pp["invcnt"][None, :]
        gconst[:, 2, :] = pp["hasg"][None, :]
        in_maps.append({
            "xT": np.ascontiguousarray(pp["xT"][c]),
            "eidx": eidx,
            "pidx": pidx,
            "consts": consts,
            "gconst": gconst,
            "encW": fw["enc_W"].astype(np.float16),
            "encb": fw["enc_b"][None, :].astype(np.float16),
            "ABp": fw["ABp"].astype(np.float16),
            "B": fw["B_bd"].astype(np.float16),
            "Wxb": fw["Wxb"].astype(np.float16),
            "Wzp": fw["Wzp"].astype(np.float16),
            "hW1": np.ascontiguousarray(fw["out_W1"].reshape(3, 5, H, H)),
            "hb1": fw["out_b1"][None, :],
            "hW2": fw["out_W2"],
            "hb2": fw["out_b2"].reshape(1, 1),
            "negrow": np.full((1, 128), BIGNEG, np.float16),
        })

    trace = bool(int(os.environ.get("KERNEL_TRACE", "0")))
    res = run_bass_kernel_spmd(nc, in_maps, core_ids=list(range(NC)), trace=trace)
    if trace and res.exec_time_ns is not None:
        print(f"HW exec time: {res.exec_time_ns} ns")
    out = np.asarray(res.results[0]["out"], np.float32).reshape(G, 1)
    return out
